# revision 1
# baseline (speedup 1.0000x reference)
"""AttentionWithBias (AlphaFold-style gated attention with pair bias) on 8 trn2 cores.

Sharding: core c handles batch b = c//4, query block qb = c%4 (128 queries).
Each core streams its [128, 512, 128] f32 bias slice ONCE, as a host-side
pre-transposed bf16 copy [d, k, q] (16.8 MB/core — half the HBM traffic of
the two-copy v1 scheme), in 4 key-chunks of 128 split into 32-key quarters.

Per 32-key quarter (pipelined DMA -> square -> PE -> stats):
  - square the quarter on DVE/ACT (alternating) -> sq
  - per k: PE matmul lhsT=chunk[:, k, :] (128-col FWL weight load),
    rhs=wext[:, 0:9] -> raw'[q, 9] (cols 0..7 = heads through the
    mean-centered g*Wb, col 8 = mean); a second matmul lhsT=sq[:, k, :],
    rhs=ones lands sumsq in col 9 of the same PSUM piece.  All LayerNorm
    statistics come out of the PE — no DVE reduction tree, no partition
    reduction, no extra HBM pass.
  - per-quarter rinv = exp(-.5*ln(sumsq/128 - mean^2 + eps)) releases the
    PSUM piece early; t1 = piece * rinv is read contiguously from PSUM.
Per chunk: t2 = t1 + S on the 8 head cols (GPSIMD, strided — cols 8..15 are
never read downstream), p = exp(t2) contiguously on ACT, PE is_transpose
flips p per head, and PV accumulates into one persistent PSUM bank with an
appended ones column on v producing the softmax denominators for free.
start_tensor_calc is only set on the very first PV matmul: on HW it marks
the whole 2KB bank pending-zero, so a per-head start would wipe earlier
heads' accumulation.

PE program order is pipelined by hand (the PE queue is strictly in-order):
proj/ss(chunk 0) first, then the phase-0 QKV/gate/logit matmuls (they wait
~10us on LayerNorm), then proj/ss(ci+1) ahead of transposes/PV(ci).

Per-(q,h)-constant terms cancel in softmax (c2, query-side mask); fully
masked query rows are zeroed by the final row mask.  Measured: 124.5 us
(NTFF, 8-core SPMD) vs 187.8 us for the v1 two-upload/DVE-tree kernel;
max rel err vs the fp32 reference 5.6e-3 (bf16-dominated).
"""

import sys

if "/opt/trn_rl_repo" not in sys.path:
    sys.path.insert(0, "/opt/trn_rl_repo")

from contextlib import ExitStack

import ml_dtypes
import numpy as np

import concourse.bacc as bacc
import concourse.bass as bass
import concourse.tile as tile
from concourse import masks, mybir
from concourse.bass_utils import run_bass_kernel_spmd

BF16 = ml_dtypes.bfloat16
F32 = mybir.dt.float32
BF = mybir.dt.bfloat16
AF = mybir.ActivationFunctionType
OP = mybir.AluOpType

D_IN = 256
D_BIAS = 128
H = 8
DH = 32
B = 2
L = 512
SCALE = 1.0 / np.sqrt(DH)
QB = 128          # queries per core
KC = 128          # keys per streamed chunk
NCH = L // KC     # chunks
NEG = -2.0e9
EPS = 1e-5

_CACHE = {}


def _ap(base, off, dims):
    return bass.AP(tensor=base.tensor, offset=base.offset + off, ap=[list(base.ap[0])] + dims)


def _build():
    nc = bacc.Bacc("TRN2", target_bir_lowering=False, debug=False, num_devices=8)

    bias_tr = nc.declare_dram_parameter("bias_tr", [D_BIAS, L, QB], BF, isOutput=False)
    x_b = nc.declare_dram_parameter("x_b", [L, D_IN], F32, isOutput=False)
    x_q = nc.declare_dram_parameter("x_q", [QB, D_IN], F32, isOutput=False)
    mk = nc.declare_dram_parameter("mk", [128, L], F32, isOutput=False)
    rowm = nc.declare_dram_parameter("rowm", [128, 1], F32, isOutput=False)
    wext = nc.declare_dram_parameter("wext", [D_BIAS, 16], BF, isOutput=False)
    # projection weights stacked host-side as [128, 5, 2, 256]: one DMA issue
    wall = nc.declare_dram_parameter("wall", [128, 5, 2, D_IN], BF, isOutput=False)
    # per-projection row biases [1, 256] (ln_in_b folded through each W, + bg for gate)
    brows = nc.declare_dram_parameter("brows", [5, D_IN], BF, isOutput=False)

    out = nc.declare_dram_parameter("out", [QB, D_IN], F32, isOutput=True)

    with tile.TileContext(nc) as tc, ExitStack() as ctx:
        sing = ctx.enter_context(tc.tile_pool(name="sing", bufs=1))
        ldp = ctx.enter_context(tc.tile_pool(name="ldp", bufs=2))
        sqp = ctx.enter_context(tc.tile_pool(name="sqp", bufs=2))
        scr = ctx.enter_context(tc.tile_pool(name="scr", bufs=2))
        ptp = ctx.enter_context(tc.tile_pool(name="ptp", bufs=2))
        lnp = ctx.enter_context(tc.tile_pool(name="lnp", bufs=5))
        ps_raw = ctx.enter_context(tc.tile_pool(name="ps_raw", bufs=4, space="PSUM"))
        ps_pt = ctx.enter_context(tc.tile_pool(name="ps_pt", bufs=1, space="PSUM"))
        ps_pv = ctx.enter_context(tc.tile_pool(name="ps_pv", bufs=1, space="PSUM"))

        def ps_tile():
            return ps_raw.tile([128, 512], F32, tag="rawps", name="rawps")

        # ---------------- phase 0: small tensors ----------------
        wext_sb = sing.tile([D_BIAS, 16], BF)
        nc.sync.dma_start(out=wext_sb[:], in_=wext[:, :])
        wall_sb = sing.tile([128, 5, 2, D_IN], BF)
        nc.sync.dma_start(out=wall_sb[:], in_=wall[:, :, :, :])
        w_sb = {n: wall_sb[:, i] for i, n in enumerate("qkvgo")}
        brow_sb = sing.tile([1, 5, D_IN], BF)
        nc.sync.dma_start(out=brow_sb[:], in_=brows[None, :, :])
        ones_row = sing.tile([1, L], BF)
        nc.vector.memset(ones_row[:], 1.0)
        ones_col = sing.tile([128, 1], BF)
        nc.vector.memset(ones_col[:], 1.0)
        mk_sb = sing.tile([128, L], F32)
        nc.sync.dma_start(out=mk_sb[:], in_=mk[:, :])
        rowm_sb = sing.tile([128, 1], F32)
        nc.sync.dma_start(out=rowm_sb[:], in_=rowm[:, :])
        eps_sb = sing.tile([128, 1], F32)
        nc.vector.memset(eps_sb[:], EPS)
        ident = sing.tile([128, 128], BF)
        masks.make_identity(nc, ident[:])

        # ---- LayerNorm(x) -> xn (bf16), for all 512 rows + the q block ----
        xall = sing.tile([128, 4, D_IN], F32)
        nc.sync.dma_start(out=xall[:], in_=x_b[:, :].rearrange("(r p) c -> p r c", p=128))

        def ln_rows(dst_ap, xt, tag):
            st6 = lnp.tile([128, 6], F32, tag="ln_st6")
            nc.vector.bn_stats(out=st6[:], in_=xt)
            mv = lnp.tile([128, 2], F32, tag="ln_mv")
            nc.vector.bn_aggr(out=mv[:], in_=st6[:])
            # rstd = exp(-0.5*ln(var+eps)) — keeps ACT inside one table set
            s = lnp.tile([128, 2], F32, tag="ln_s")
            nc.scalar.activation(s[:, 0:1], mv[:, 1:2], AF.Ln, bias=eps_sb[:, 0:1])
            nc.scalar.activation(s[:, 1:2], s[:, 0:1], AF.Exp, scale=-0.5)
            nc.vector.tensor_scalar(
                out=dst_ap, in0=xt, scalar1=mv[:, 0:1], scalar2=s[:, 1:2],
                op0=OP.subtract, op1=OP.mult,
            )

        xn_sb = sing.tile([128, 4, D_IN], BF)
        for r in range(4):
            ln_rows(xn_sb[:, r, :], xall[:, r, :], f"xr{r}")
        xq_sb = sing.tile([128, D_IN], BF)
        xqt = lnp.tile([128, D_IN], F32, tag="ln_x")
        nc.sync.dma_start(out=xqt[:], in_=x_q[:, :])
        ln_rows(xq_sb[:], xqt[:], "xq")

        # ---- transposes: xnT [din-chunk, 512 rows], xqT [din-chunk, 128] ----
        xnT = sing.tile([128, 2, L], BF)
        for r in range(4):
            nc.scalar.dma_start_transpose(xnT[:, :, r * 128:(r + 1) * 128], xn_sb[:, r, :])
        xqT = sing.tile([128, 2, QB], BF)
        nc.scalar.dma_start_transpose(xqT[:], xq_sb[:])

        P0 = {}

        def emit_phase0_mms():
            # ---- kT, qT ----
            kT = sing.tile([128, 2, L], BF)
            for h2 in range(2):
                pk = ps_tile()
                nc.tensor.matmul(pk[:], lhsT=w_sb["k"][:, 0, h2 * 128:(h2 + 1) * 128],
                                 rhs=xnT[:, 0, :], start=True, stop=False)
                nc.tensor.matmul(pk[:], lhsT=w_sb["k"][:, 1, h2 * 128:(h2 + 1) * 128],
                                 rhs=xnT[:, 1, :], start=False, stop=False)
                nc.tensor.matmul(pk[:], lhsT=brow_sb[:, 1, h2 * 128:(h2 + 1) * 128],
                                 rhs=ones_row[:], start=False, stop=True)
                nc.scalar.copy(kT[:, h2, :], pk[:])
            qT = sing.tile([128, 2, QB], BF)
            for h2 in range(2):
                pq = ps_tile()[:, 0:QB]
                nc.tensor.matmul(pq[:], lhsT=w_sb["q"][:, 0, h2 * 128:(h2 + 1) * 128],
                                 rhs=xqT[:, 0, :], start=True, stop=False)
                nc.tensor.matmul(pq[:], lhsT=w_sb["q"][:, 1, h2 * 128:(h2 + 1) * 128],
                                 rhs=xqT[:, 1, :], start=False, stop=False)
                nc.tensor.matmul(pq[:], lhsT=brow_sb[:, 0, h2 * 128:(h2 + 1) * 128],
                                 rhs=ones_row[:, 0:QB], start=False, stop=True)
                nc.scalar.copy(qT[:, h2, :], pq[:])

            # ---- v_ext [k%128, kchunk, h, 33]: v with a ones column per head ----
            v_ext = sing.tile([128, 4, H, 33], BF)
            nc.vector.memset(v_ext[:], 1.0)
            for r in range(4):
                pv = ps_tile()[:, 0:D_IN]
                nc.tensor.matmul(pv[:], lhsT=xnT[:, 0, r * 128:(r + 1) * 128],
                                 rhs=w_sb["v"][:, 0, :], start=True, stop=False)
                nc.tensor.matmul(pv[:], lhsT=xnT[:, 1, r * 128:(r + 1) * 128],
                                 rhs=w_sb["v"][:, 1, :], start=False, stop=False)
                nc.tensor.matmul(pv[:], lhsT=ones_row[:, 0:128],
                                 rhs=brow_sb[:, 2, :], start=False, stop=True)
                nc.vector.tensor_copy(v_ext[:, r, :, 0:32], pv[:].rearrange("p (h d) -> p h d", h=H))

            # ---- gate = sigmoid(xq @ Wg + bgate) ----
            gate_sb = sing.tile([128, D_IN], F32)
            pg = ps_tile()[:, 0:D_IN]
            nc.tensor.matmul(pg[:], lhsT=xqT[:, 0, :], rhs=w_sb["g"][:, 0, :],
                             start=True, stop=False)
            nc.tensor.matmul(pg[:], lhsT=xqT[:, 1, :], rhs=w_sb["g"][:, 1, :],
                             start=False, stop=False)
            nc.tensor.matmul(pg[:], lhsT=ones_row[:, 0:128], rhs=brow_sb[:, 3, :],
                             start=False, stop=True)
            # sigmoid(x) = 1/(1+exp(-x)) — avoids loading the sigmoid ACT table set
            nc.scalar.activation(gate_sb[:], pg[:], AF.Exp, scale=-1.0)
            nc.vector.tensor_scalar(out=gate_sb[:], in0=gate_sb[:], scalar1=1.0,
                                    scalar2=None, op0=OP.add)
            nc.vector.reciprocal(gate_sb[:], gate_sb[:])

            # ---- S[q, h, k] = qk logits + key mask ----
            s_all = sing.tile([128, H, L], F32)
            for h in range(H):
                pS = ps_tile()
                base = 32 * (h % 4)
                nc.tensor.matmul(pS[:], lhsT=qT[base:base + 32, h // 4, :],
                                 rhs=kT[base:base + 32, h // 4, :],
                                 start=True, stop=True, tile_position=(base, 0))
                nc.vector.tensor_tensor(out=s_all[:, h, :], in0=pS[:], in1=mk_sb[:], op=OP.add)

            P0.update(kT=kT, qT=qT, v_ext=v_ext, gate_sb=gate_sb, s_all=s_all)

        # ---------------- phase 1: stream bias chunks ----------------
        # PE queue is in-order: emit chunk-0 proj before the phase-0 QKV
        # matmuls (those wait ~30us on LayerNorm + transposes), and emit
        # proj(ci+1) before transposes/PV(ci) so the PE never idles while
        # the fixup chain (rinv -> t1 -> t2 -> exp) of chunk ci drains.
        pvps = ps_pv.tile([128, H * 33], F32)

        def emit_chunk_front(ci):
            """DMA + squares + proj/ss matmuls + per-quarter stats/rinv/t1."""
            tbs = []
            for g in range(4):
                tbg = ldp.tile([128, 32, QB], BF, tag=f"tb{g}", name=f"tb{g}")
                nc.sync.dma_start(out=tbg[:],
                                  in_=bias_tr[:, ci * KC + g * 32:ci * KC + (g + 1) * 32, :])
                tbs.append(tbg)
            t1 = scr.tile([128, KC * 16], F32, tag="fx1", name="t1")
            for g in range(4):
                tbg = tbs[g]
                sqg = sqp.tile([128, 32, QB], BF, tag=f"sq{g}", name=f"sq{g}")
                # last chunk: ACT takes the early quarters, DVE the late ones —
                # the drain critical path runs through the last quarters' squares
                if ci == 0:
                    use_dve = True      # ACT is busy with LN + xbar transposes here
                elif ci < NCH - 1:
                    use_dve = (g % 2 == 0)
                else:
                    use_dve = (g >= 2)  # drain: DVE takes the late quarters
                for hh in range(2):
                    sl = slice(hh * 16, (hh + 1) * 16)
                    if use_dve:
                        nc.vector.tensor_tensor(out=sqg[:, sl, :], in0=tbg[:, sl, :],
                                                in1=tbg[:, sl, :], op=OP.mult)
                    else:
                        nc.scalar.activation(sqg[:, sl, :], tbg[:, sl, :], AF.Square)
                rp = ps_tile()
                # all projections first: they depend only on the DMA, while the
                # sumsq matmuls wait for the square -- keeps the in-order PE fed
                for j in range(32):
                    nc.tensor.matmul(rp[:, j * 16:j * 16 + 9], lhsT=tbg[:, j, :],
                                     rhs=wext_sb[:, 0:9], start=True, stop=True)
                for j in range(32):
                    nc.tensor.matmul(rp[:, j * 16 + 9:j * 16 + 10], lhsT=sqg[:, j, :],
                                     rhs=ones_col[:], start=True, stop=True)
                msq = scr.tile([128, 32], F32, tag="msq", name="msq")
                nc.scalar.activation(msq[:], _ap(rp[:], 8, [[16, 32]]), AF.Square)
                var_g = scr.tile([128, 32], F32, tag="var", name="var_g")
                nc.vector.scalar_tensor_tensor(out=var_g[:],
                                               in0=_ap(rp[:], 9, [[16, 32]]),
                                               scalar=1.0 / D_BIAS, in1=msq[:],
                                               op0=OP.mult, op1=OP.subtract)
                # per-quarter rinv so the PSUM piece is released quickly
                lnv = scr.tile([128, 32], F32, tag="lnv", name="lnv")
                nc.scalar.activation(lnv[:], var_g[:], AF.Ln, bias=eps_sb[:, 0:1])
                rinv_g = scr.tile([128, 32], F32, tag="rinv", name="rinv_g")
                nc.scalar.activation(rinv_g[:], lnv[:], AF.Exp, scale=-0.5)
                nc.vector.tensor_tensor(
                    out=t1[:, g * 512:(g + 1) * 512].rearrange("p (k c) -> p k c", c=16),
                    in0=rp[:].rearrange("p (k c) -> p k c", c=16),
                    in1=_ap(rinv_g[:], 0, [[1, 32], [0, 16]]), op=OP.mult)
            return (t1,)

        def emit_chunk_fixup(ci, t1):
            """t2 = t1 + S (GPSIMD, strided on head cols) -> exp (ACT)."""
            p_sb = scr.tile([128, KC * 16], BF, tag="p", name="p_sb")
            nseg = 2 if ci < NCH - 1 else 4   # finer drain on the last chunk
            kseg = KC // nseg
            for hf in range(nseg):
                o = hf * kseg * 16
                nc.gpsimd.tensor_tensor(
                    out=_ap(t1[:], o, [[16, kseg], [1, H]]),
                    in0=_ap(t1[:], o, [[16, kseg], [1, H]]),
                    in1=_ap(P0['s_all'][:], ci * KC + hf * kseg, [[1, kseg], [L, H]]),
                    op=OP.add)
                nc.scalar.activation(p_sb[:, o:o + kseg * 16], t1[:, o:o + kseg * 16], AF.Exp)
            return p_sb

        def emit_chunk_back(ci, p_sb):
            """transposes + PV accumulation (PE)."""
            pT_ps = ps_pt.tile([128, H, 128], BF, tag="ptps", name="pT_ps")
            for h in range(H):
                nc.tensor.transpose(pT_ps[:, h, :], _ap(p_sb[:], h, [[16, KC]]), ident[:])
            pT_sb = ptp.tile([128, H, 128], BF, tag="ptsb", name="pT_sb")
            nc.vector.tensor_copy(pT_sb[:], pT_ps[:])
            for h in range(H):
                # start only on the very first matmul into the bank: on HW,
                # start_tensor_calc marks the whole 2KB bank pending-zero, so a
                # per-head start would wipe earlier heads' accumulation.
                nc.tensor.matmul(pvps[:, h * 33:(h + 1) * 33], lhsT=pT_sb[:, h, :],
                                 rhs=P0['v_ext'][:, ci, h, :],
                                 start=(ci == 0 and h == 0), stop=(ci == NCH - 1))

        state = emit_chunk_front(0)
        emit_phase0_mms()
        pending = (0, emit_chunk_fixup(0, *state))
        for ci in range(1, NCH):
            state = emit_chunk_front(ci)
            emit_chunk_back(*pending)
            pending = (ci, emit_chunk_fixup(ci, *state))
        emit_chunk_back(*pending)

        # ---------------- phase 2: denominators, gate, output ----------------
        dn = sing.tile([128, 16], F32)
        nc.vector.tensor_scalar(out=dn[:, 0:8], in0=_ap(pvps[:], 32, [[33, 8]]),
                                scalar1=1e-30, scalar2=None, op0=OP.add)
        nc.vector.reciprocal(dn[:, 8:16], dn[:, 0:8])

        comb1 = sing.tile([128, D_IN], F32)
        nc.vector.tensor_tensor(out=comb1[:].rearrange("p (h d) -> p h d", h=H),
                                in0=_ap(pvps[:], 0, [[33, 8], [1, 32]]),
                                in1=P0['gate_sb'][:].rearrange("p (h d) -> p h d", h=H),
                                op=OP.mult)
        comb = sing.tile([128, D_IN], BF)
        nc.vector.tensor_tensor(out=comb[:].rearrange("p (h d) -> p h d", h=H),
                                in0=comb1[:].rearrange("p (h d) -> p h d", h=H),
                                in1=_ap(dn[:], 8, [[1, 8], [0, DH]]), op=OP.mult)

        cT_ps = ps_pt.tile([128, H, 128], BF, tag="ptps")
        for c in range(2):
            nc.tensor.transpose(cT_ps[:, c, :], comb[:, c * 128:(c + 1) * 128], ident[:])
        cT_sb = ptp.tile([128, 2, 128], BF, tag="ctsb")
        nc.vector.tensor_copy(cT_sb[:], cT_ps[:, 0:2, :])

        fin = ps_tile()[:, 0:D_IN]
        for c in range(2):
            nc.tensor.matmul(fin[:], lhsT=cT_sb[:, c, :], rhs=w_sb["o"][:, c, :],
                             start=(c == 0), stop=False)
        nc.tensor.matmul(fin[:], lhsT=ones_row[:, 0:128], rhs=brow_sb[:, 4, :],
                         start=False, stop=True)
        out_sb = sing.tile([128, D_IN], F32)
        nc.scalar.activation(out_sb[:], fin[:], AF.Copy, scale=rowm_sb[:, 0:1])
        nc.sync.dma_start(out=out[:, :], in_=out_sb[:])

    # Steer insert_act_table_loads to the one set that covers Square/Ln/Exp/Copy
    # (otherwise it alternates exp_and_others <-> natural_log, ~19 table loads).
    orig_tables = bacc.get_activation_tables
    keep = "natural_log_exp_and_others"

    def _patched(arch):
        t = orig_tables(arch)
        return {name: (fs if name == keep else set()) for name, fs in t.items()}

    bacc.get_activation_tables = _patched
    try:
        nc.compile()
    finally:
        bacc.get_activation_tables = orig_tables
    return nc


def _prep_common(inputs):
    f32 = np.float32
    ln_in_g = np.asarray(inputs["ln_in_g"], np.float64)
    ln_in_b = np.asarray(inputs["ln_in_b"], np.float64)
    ln_b_g = np.asarray(inputs["ln_b_g"], np.float64)
    Wq = np.asarray(inputs["Wq"], np.float64)
    Wk = np.asarray(inputs["Wk"], np.float64)
    Wv = np.asarray(inputs["Wv"], np.float64)
    Wg = np.asarray(inputs["Wg"], np.float64)
    Wb = np.asarray(inputs["Wb"], np.float64)
    Wo = np.asarray(inputs["Wo"], np.float64)
    bg = np.asarray(inputs["bg"], np.float64)
    bo = np.asarray(inputs["bo"], np.float64)

    def arr_w(w):  # [256, 256] -> [128, 2, 256] din-chunk grouping
        return np.ascontiguousarray(
            w.reshape(2, 128, D_IN).transpose(1, 0, 2)).astype(BF16)

    wq_e = arr_w(Wq * ln_in_g[:, None])
    wk_e = arr_w(Wk * ln_in_g[:, None] * SCALE)
    wv_e = arr_w(Wv * ln_in_g[:, None])
    wg_e = arr_w(Wg * ln_in_g[:, None])
    wo_e = arr_w(Wo)

    brows = np.stack([
        ln_in_b @ Wq,
        (ln_in_b @ Wk) * SCALE,
        ln_in_b @ Wv,
        ln_in_b @ Wg + bg,
        bo,
    ]).astype(BF16)

    c1 = ln_b_g @ Wb                        # [H]
    wext = np.zeros((D_BIAS, 16), np.float64)
    # head cols pre-centered: T @ (g*Wb - c1/128) == T@ (g*Wb) - mean(T)*c1
    wext[:, 0:H] = Wb * ln_b_g[:, None] - c1[None, :] / D_BIAS
    wext[:, 8] = 1.0 / D_BIAS
    wext = wext.astype(BF16)

    wall = np.ascontiguousarray(np.stack([wq_e, wk_e, wv_e, wg_e, wo_e], axis=1))
    return dict(wall=wall, brows=brows, wext=wext)


def _make_in_maps(inputs):
    x = np.asarray(inputs["x"], np.float32)
    bias = np.asarray(inputs["bias"], np.float32)
    mask = np.asarray(inputs["mask"])
    common = _prep_common(inputs)

    in_maps = []
    for c in range(8):
        b, qb = divmod(c, 4)
        q0 = qb * QB
        mrow = (mask[b] == 0).astype(np.float32) * NEG          # [512]
        mk_bc = np.broadcast_to(mrow, (128, L)).copy()
        rowm = (mask[b, q0:q0 + QB] != 0).astype(np.float32)[:, None].copy()
        nat = bias[b, q0:q0 + QB].astype(BF16)
        in_maps.append(dict(
            bias_tr=np.ascontiguousarray(nat.transpose(2, 1, 0)),
            x_b=x[b],
            x_q=np.ascontiguousarray(x[b, q0:q0 + QB]),
            mk=mk_bc,
            rowm=rowm,
            **common,
        ))
    return in_maps


def kernel(**inputs):
    if "nc" not in _CACHE:
        _CACHE["nc"] = _build()
    nc = _CACHE["nc"]

    in_maps = _make_in_maps(inputs)
    res = run_bass_kernel_spmd(nc, in_maps, list(range(8)))
    out = np.empty((B, L, D_IN), np.float32)
    for c in range(8):
        b, qb = divmod(c, 4)
        out[b, qb * QB:(qb + 1) * QB] = res.results[c]["out"]
    return out



# revision 11
# speedup vs baseline: 1.1442x; 1.1442x over previous
"""AttentionWithBias (AlphaFold-style gated attention with pair bias) on 8 trn2 cores.

v2: mask compaction. mask==0 kills whole key columns (p=exp(-1e9-..)==0 exactly
in f32) and whole query rows (output zeroed), so kernel() compacts both host-side:
each core gets only the unmasked keys of its batch (padded to a multiple of 32,
uniform across cores) and a ~1/4 share of the unmasked queries (padded to mult 32).
For the seed-0 input that is kpad=256 keys, qpad=64 queries -> the per-core bias
slice is [128, 256, 64] bf16 = 4.2 MB (4x less HBM than v1), which fits whole in
SBUF: all bias DMAs are issued up front, so the DMA never stalls on compute.

Pipeline (per core): LN(x_keys)/LN(x_q) -> xnT/xqT (xbar transposes); per 32-key
quarter: square (DVE, 2 quarters on ACT), per-key PE pair {proj: lhsT=bias col
block [128d, qpad], rhs=wext[:, 0:9] -> [q, 9] (8 heads pre-centered + mean);
sumsq: lhsT=sq, rhs=ones}, pitch 10 cols per key in PSUM; per-quarter stats
rinv=exp(-.5 ln(sumsq/128 - mean^2 + eps)) then t1 = piece*rinv releases PSUM.
Phase0 (kT/qT/v_ext/gate/S logits) sits between chunk-0 and chunk-1 projections
in PE program order. Fixup: t2 = t1 + S on DVE (strided head cols), p = exp(t2)
on ACT, PE transposes p per head, PV accumulates into one persistent PSUM bank
with an appended ones column giving softmax denominators. Bias-row matmuls are
elided when the folded row biases are zero (they are: ln_in_b=bo=0); the gate
bias bg is uniform so it folds into the sigmoid's activation bias operand.

Padded keys are masked via mk=-2e9; padded/garbage query rows stay confined to
partitions >= qpad (matmul contractions never read them) and are never DMA'd out.
"""

import sys

if "/opt/trn_rl_repo" not in sys.path:
    sys.path.insert(0, "/opt/trn_rl_repo")

from contextlib import ExitStack

import ml_dtypes
import numpy as np

import concourse.bacc as bacc
import concourse.bass as bass
import concourse.tile as tile
from concourse import masks, mybir
from concourse.bass_utils import run_bass_kernel_spmd

BF16 = ml_dtypes.bfloat16
F32 = mybir.dt.float32
BF = mybir.dt.bfloat16
AF = mybir.ActivationFunctionType
OP = mybir.AluOpType

D_IN = 256
D_BIAS = 128
H = 8
DH = 32
B = 2
L = 512
SCALE = 1.0 / np.sqrt(DH)
NEG = -2.0e9
EPS = 1e-5
P = 10            # PSUM column pitch per key: 8 heads + mean + sumsq

_CACHE = {}


def _ap(base, off, dims):
    return bass.AP(tensor=base.tensor, offset=base.offset + off, ap=[list(base.ap[0])] + dims)


def _build(kpad, qpad, modes, nq_cores):
    """modes: per-projection bias handling for (q, k, v, g, o):
    0 = no bias row (all zero), 1 = uniform scalar (gate only; via act bias), 2 = matmul."""
    nkr = (kpad + 127) // 128
    kfull = nkr * 128
    chunks = []
    rem = kpad
    while rem > 0:
        chunks.append(min(128, rem))
        rem -= 128
    NCH = len(chunks)

    nc = bacc.Bacc("TRN2", target_bir_lowering=False, debug=False, num_devices=8)

    bias_tr = nc.declare_dram_parameter("bias_tr", [D_BIAS, kpad, qpad], BF, isOutput=False)
    x_k = nc.declare_dram_parameter("x_k", [kfull, D_IN], F32, isOutput=False)
    x_q = nc.declare_dram_parameter("x_q", [qpad, D_IN], F32, isOutput=False)
    mk = nc.declare_dram_parameter("mk", [128, kpad], F32, isOutput=False)
    rowm = nc.declare_dram_parameter("rowm", [128, 1], F32, isOutput=False)
    wext = nc.declare_dram_parameter("wext", [D_BIAS, 16], BF, isOutput=False)
    wall = nc.declare_dram_parameter("wall", [128, 5, 2, D_IN], BF, isOutput=False)
    brows = nc.declare_dram_parameter("brows", [5, D_IN], BF, isOutput=False)
    gbias = nc.declare_dram_parameter("gbias", [128, 1], F32, isOutput=False)

    out = nc.declare_dram_parameter("out", [qpad, D_IN], F32, isOutput=True)

    need_ones_row = any(m == 2 for m in modes)

    with tile.TileContext(nc) as tc, ExitStack() as ctx:
        sing = ctx.enter_context(tc.tile_pool(name="sing", bufs=1))
        scr = ctx.enter_context(tc.tile_pool(name="scr", bufs=2))
        ptp = ctx.enter_context(tc.tile_pool(name="ptp", bufs=2))
        lnp = ctx.enter_context(tc.tile_pool(name="lnp", bufs=5))
        ps_raw = ctx.enter_context(tc.tile_pool(name="ps_raw", bufs=5, space="PSUM"))
        ps_pt = ctx.enter_context(tc.tile_pool(name="ps_pt", bufs=1, space="PSUM"))
        ps_pv = ctx.enter_context(tc.tile_pool(name="ps_pv", bufs=1, space="PSUM"))

        def ps_tile():
            # full 2KB bank per piece: matmul outputs must not cross bank bounds
            return ps_raw.tile([128, 512], F32, tag="rawps", name="rawps")

        # ---------------- small DMAs first (needed earliest) ----------------
        wext_sb = sing.tile([D_BIAS, 16], BF)
        nc.sync.dma_start(out=wext_sb[:], in_=wext[:, :])
        wall_sb = sing.tile([128, 5, 2, D_IN], BF)
        nc.sync.dma_start(out=wall_sb[:], in_=wall[:, :, :, :])
        w_sb = {n: wall_sb[:, i] for i, n in enumerate("qkvgo")}
        brow_sb = sing.tile([1, 5, D_IN], BF)
        nc.sync.dma_start(out=brow_sb[:], in_=brows[None, :, :])
        mk_sb = sing.tile([128, kpad], F32)
        nc.sync.dma_start(out=mk_sb[:], in_=mk[:, :])
        rowm_sb = sing.tile([128, 1], F32)
        nc.sync.dma_start(out=rowm_sb[:], in_=rowm[:, :])
        gb_sb = sing.tile([128, 1], F32)
        nc.sync.dma_start(out=gb_sb[:], in_=gbias[:, :])
        xqt = lnp.tile([128, D_IN], F32, tag="ln_x")
        nc.sync.dma_start(out=xqt[0:qpad, :], in_=x_q[:, :])
        xall = sing.tile([128, nkr, D_IN], F32)
        nc.sync.dma_start(out=xall[:], in_=x_k[:, :].rearrange("(r p) c -> p r c", p=128))

        # ---------------- all bias quarter DMAs up front ----------------
        tbs = {}
        for ci, csz in enumerate(chunks):
            for g in range(csz // 32):
                k0 = ci * 128 + g * 32
                tbg = sing.tile([128, 32, qpad], BF, name=f"tb{ci}_{g}")
                nc.sync.dma_start(out=tbg[:], in_=bias_tr[:, k0:k0 + 32, :])
                tbs[(ci, g)] = tbg

        # ---------------- constants ----------------
        if need_ones_row:
            ones_row = sing.tile([1, max(kpad, D_IN)], BF)
            nc.vector.memset(ones_row[:], 1.0)
        ones_col = sing.tile([128, 1], BF)
        nc.vector.memset(ones_col[:], 1.0)
        eps_sb = sing.tile([128, 1], F32)
        nc.vector.memset(eps_sb[:], EPS)
        ident = sing.tile([128, 128], BF)
        masks.make_identity(nc, ident[:])

        # ---------------- LayerNorm(x) ----------------
        def ln_rows(dst_ap, xt):
            st6 = lnp.tile([128, 6], F32, tag="ln_st6")
            nc.vector.bn_stats(out=st6[:], in_=xt)
            mv = lnp.tile([128, 2], F32, tag="ln_mv")
            nc.vector.bn_aggr(out=mv[:], in_=st6[:])
            s = lnp.tile([128, 2], F32, tag="ln_s")
            nc.scalar.activation(s[:, 0:1], mv[:, 1:2], AF.Ln, bias=eps_sb[:, 0:1])
            nc.scalar.activation(s[:, 1:2], s[:, 0:1], AF.Exp, scale=-0.5)
            nc.vector.tensor_scalar(
                out=dst_ap, in0=xt, scalar1=mv[:, 0:1], scalar2=s[:, 1:2],
                op0=OP.subtract, op1=OP.mult,
            )

        xn_sb = sing.tile([128, nkr, D_IN], BF)
        for r in range(nkr):
            ln_rows(xn_sb[:, r, :], xall[:, r, :])
        xq_sb = sing.tile([128, D_IN], BF)
        ln_rows(xq_sb[:], xqt[:])

        xnT = sing.tile([128, 2, kfull], BF)
        for r in range(nkr):
            nc.scalar.dma_start_transpose(xnT[:, :, r * 128:(r + 1) * 128], xn_sb[:, r, :])
        xqT = sing.tile([128, 2, 128], BF)
        nc.scalar.dma_start_transpose(xqT[:], xq_sb[:])

        # ---------------- squares (DVE; a couple on ACT for balance) ----------------
        sqs = {}
        for ci, csz in enumerate(chunks):
            for g in range(csz // 32):
                tbg = tbs[(ci, g)]
                sqg = sing.tile([128, 32, qpad], BF, name=f"sq{ci}_{g}")
                use_act = (ci >= 1 and g in (1, 3))
                for hh in range(2):
                    sl = slice(hh * 16, (hh + 1) * 16)
                    if use_act:
                        nc.scalar.activation(sqg[:, sl, :], tbg[:, sl, :], AF.Square)
                    else:
                        nc.vector.tensor_tensor(out=sqg[:, sl, :], in0=tbg[:, sl, :],
                                                in1=tbg[:, sl, :], op=OP.mult)
                sqs[(ci, g)] = sqg

        P0 = {}

        def emit_phase0_mms():
            kT = sing.tile([128, 2, kpad], BF)
            for h2 in range(2):
                pk = ps_tile()[:, 0:kpad]
                nc.tensor.matmul(pk[:], lhsT=w_sb["k"][:, 0, h2 * 128:(h2 + 1) * 128],
                                 rhs=xnT[:, 0, 0:kpad], start=True, stop=(modes[1] != 2))
                nc.tensor.matmul(pk[:], lhsT=w_sb["k"][:, 1, h2 * 128:(h2 + 1) * 128],
                                 rhs=xnT[:, 1, 0:kpad], start=False, stop=(modes[1] != 2))
                if modes[1] == 2:
                    nc.tensor.matmul(pk[:], lhsT=brow_sb[:, 1, h2 * 128:(h2 + 1) * 128],
                                     rhs=ones_row[:, 0:kpad], start=False, stop=True)
                nc.scalar.copy(kT[:, h2, :], pk[:])
            qT = sing.tile([128, 2, qpad], BF)
            for h2 in range(2):
                pq = ps_tile()[:, 0:qpad]
                nc.tensor.matmul(pq[:], lhsT=w_sb["q"][:, 0, h2 * 128:(h2 + 1) * 128],
                                 rhs=xqT[:, 0, 0:qpad], start=True, stop=(modes[0] != 2))
                nc.tensor.matmul(pq[:], lhsT=w_sb["q"][:, 1, h2 * 128:(h2 + 1) * 128],
                                 rhs=xqT[:, 1, 0:qpad], start=False, stop=(modes[0] != 2))
                if modes[0] == 2:
                    nc.tensor.matmul(pq[:], lhsT=brow_sb[:, 0, h2 * 128:(h2 + 1) * 128],
                                     rhs=ones_row[:, 0:qpad], start=False, stop=True)
                nc.scalar.copy(qT[:, h2, :], pq[:])

            v_ext = sing.tile([128, nkr, H, 33], BF)
            nc.vector.memset(v_ext[:], 1.0)
            for r in range(nkr):
                pv = ps_tile()[:, 0:D_IN]
                nc.tensor.matmul(pv[:], lhsT=xnT[:, 0, r * 128:(r + 1) * 128],
                                 rhs=w_sb["v"][:, 0, :], start=True, stop=(modes[2] != 2))
                nc.tensor.matmul(pv[:], lhsT=xnT[:, 1, r * 128:(r + 1) * 128],
                                 rhs=w_sb["v"][:, 1, :], start=False, stop=(modes[2] != 2))
                if modes[2] == 2:
                    nc.tensor.matmul(pv[:], lhsT=ones_row[:, 0:128],
                                     rhs=brow_sb[:, 2, :], start=False, stop=True)
                nc.vector.tensor_copy(v_ext[:, r, :, 0:32], pv[:].rearrange("p (h d) -> p h d", h=H))

            gate_sb = sing.tile([128, D_IN], F32)
            pg = ps_tile()[0:qpad, 0:D_IN]
            nc.tensor.matmul(pg, lhsT=xqT[:, 0, 0:qpad], rhs=w_sb["g"][:, 0, :],
                             start=True, stop=(modes[3] != 2))
            nc.tensor.matmul(pg, lhsT=xqT[:, 1, 0:qpad], rhs=w_sb["g"][:, 1, :],
                             start=False, stop=(modes[3] != 2))
            if modes[3] == 2:
                nc.tensor.matmul(pg, lhsT=ones_row[:, 0:128], rhs=brow_sb[:, 3, :],
                                 start=False, stop=True)
            # sigmoid(x+b) = 1/(1+exp(-x-b)); uniform b rides the activation bias operand
            if modes[3] == 1:
                nc.scalar.activation(gate_sb[0:qpad, :], pg, AF.Exp, scale=-1.0,
                                     bias=gb_sb[0:qpad, 0:1])
            else:
                nc.scalar.activation(gate_sb[0:qpad, :], pg, AF.Exp, scale=-1.0)
            nc.vector.tensor_scalar(out=gate_sb[0:qpad, :], in0=gate_sb[0:qpad, :],
                                    scalar1=1.0, scalar2=None, op0=OP.add)
            nc.vector.reciprocal(gate_sb[0:qpad, :], gate_sb[0:qpad, :])

            s_all = sing.tile([128, H, kpad], F32)
            for h in range(H):
                pS = ps_tile()[0:qpad, 0:kpad]
                base = 32 * (h % 4)
                nc.tensor.matmul(pS, lhsT=qT[base:base + 32, h // 4, :],
                                 rhs=kT[base:base + 32, h // 4, :],
                                 start=True, stop=True, tile_position=(base, 0))
                nc.vector.tensor_tensor(out=s_all[0:qpad, h, :], in0=pS,
                                        in1=mk_sb[0:qpad, :], op=OP.add)

            P0.update(kT=kT, qT=qT, v_ext=v_ext, gate_sb=gate_sb, s_all=s_all)

        # ---------------- per-chunk front: proj/ss matmuls + stats + t1 ----------------
        pvps = ps_pv.tile([128, H * 33], F32)

        def emit_chunk_front(ci, csz):
            nq = csz // 32
            t1 = scr.tile([128, 128 * P], F32, tag="fx1", name="t1")
            for g in range(nq):
                tbg, sqg = tbs[(ci, g)], sqs[(ci, g)]
                rpt = ps_tile()
                rpb = rpt[0:qpad, 0:32 * P]
                for j in range(32):
                    nc.tensor.matmul(rpt[0:qpad, j * P:j * P + 9], lhsT=tbg[:, j, :],
                                     rhs=wext_sb[:, 0:9], start=True, stop=True)
                for j in range(32):
                    nc.tensor.matmul(rpt[0:qpad, j * P + 9:j * P + 10], lhsT=sqg[:, j, :],
                                     rhs=ones_col[:], start=True, stop=True)
                msq = scr.tile([128, 32], F32, tag="msq", name="msq")
                nc.scalar.activation(msq[0:qpad, :], _ap(rpb, 8, [[P, 32]]), AF.Square)
                var_g = scr.tile([128, 32], F32, tag="var", name="var_g")
                nc.vector.scalar_tensor_tensor(out=var_g[0:qpad, :],
                                               in0=_ap(rpb, 9, [[P, 32]]),
                                               scalar=1.0 / D_BIAS, in1=msq[0:qpad, :],
                                               op0=OP.mult, op1=OP.subtract)
                lnv = scr.tile([128, 32], F32, tag="lnv", name="lnv")
                nc.scalar.activation(lnv[0:qpad, :], var_g[0:qpad, :], AF.Ln,
                                     bias=eps_sb[0:qpad, 0:1])
                rinv_g = scr.tile([128, 32], F32, tag="rinv", name="rinv_g")
                nc.scalar.activation(rinv_g[0:qpad, :], lnv[0:qpad, :], AF.Exp, scale=-0.5)
                nc.vector.tensor_tensor(
                    out=t1[0:qpad, g * 32 * P:(g + 1) * 32 * P].rearrange(
                        "p (k c) -> p k c", c=P),
                    in0=rpb.rearrange("p (k c) -> p k c", c=P),
                    in1=_ap(rinv_g[0:qpad, :], 0, [[1, 32], [0, P]]), op=OP.mult)
            return t1

        def emit_chunk_fixup(ci, csz, t1):
            """t2 = t1 + S (DVE, strided on head cols) -> p = exp (ACT)."""
            p_sb = scr.tile([128, 128 * P], BF, tag="p", name="p_sb")
            nseg = 2 if ci < NCH - 1 else 4
            kseg = csz // nseg
            for hf in range(nseg):
                o = hf * kseg * P
                nc.vector.tensor_tensor(
                    out=_ap(t1[0:qpad, :], o, [[P, kseg], [1, H]]),
                    in0=_ap(t1[0:qpad, :], o, [[P, kseg], [1, H]]),
                    in1=_ap(P0['s_all'][0:qpad, :, :], ci * 128 + hf * kseg,
                            [[1, kseg], [kpad, H]]),
                    op=OP.add)
                nc.scalar.activation(p_sb[0:qpad, o:o + kseg * P],
                                     t1[0:qpad, o:o + kseg * P], AF.Exp)
            return p_sb

        def emit_chunk_back(ci, csz, p_sb):
            pT_ps = ps_pt.tile([128, H, 128], BF, tag="ptps", name="pT_ps")
            for h in range(H):
                nc.tensor.transpose(pT_ps[0:csz, h, :], _ap(p_sb[:], h, [[P, csz]]), ident[:])
            pT_sb = ptp.tile([128, H, 128], BF, tag="ptsb", name="pT_sb")
            nc.vector.tensor_copy(pT_sb[0:csz, :, :], pT_ps[0:csz, :, :])
            for h in range(H):
                nc.tensor.matmul(pvps[:, h * 33:(h + 1) * 33], lhsT=pT_sb[0:csz, h, :],
                                 rhs=P0['v_ext'][0:csz, ci, h, :],
                                 start=(ci == 0 and h == 0), stop=(ci == NCH - 1))

        # ---------------- main schedule ----------------
        t1 = emit_chunk_front(0, chunks[0])
        emit_phase0_mms()
        pending = (0, chunks[0], emit_chunk_fixup(0, chunks[0], t1))
        for ci in range(1, NCH):
            t1 = emit_chunk_front(ci, chunks[ci])
            emit_chunk_back(*pending)
            pending = (ci, chunks[ci], emit_chunk_fixup(ci, chunks[ci], t1))
        emit_chunk_back(*pending)

        # ---------------- phase 2: denominators, gate, output ----------------
        dn = sing.tile([128, 16], F32)
        nc.vector.tensor_scalar(out=dn[:, 0:8], in0=_ap(pvps[:], 32, [[33, 8]]),
                                scalar1=1e-30, scalar2=None, op0=OP.add)
        nc.vector.reciprocal(dn[:, 8:16], dn[:, 0:8])

        comb1 = sing.tile([128, D_IN], F32)
        nc.vector.tensor_tensor(out=comb1[:].rearrange("p (h d) -> p h d", h=H),
                                in0=_ap(pvps[:], 0, [[33, 8], [1, 32]]),
                                in1=P0['gate_sb'][:].rearrange("p (h d) -> p h d", h=H),
                                op=OP.mult)
        comb = sing.tile([128, D_IN], BF)
        nc.vector.tensor_tensor(out=comb[:].rearrange("p (h d) -> p h d", h=H),
                                in0=comb1[:].rearrange("p (h d) -> p h d", h=H),
                                in1=_ap(dn[:], 8, [[1, 8], [0, DH]]), op=OP.mult)

        cT_ps = ps_pt.tile([128, H, 128], BF, tag="ptps")
        for c in range(2):
            nc.tensor.transpose(cT_ps[:, c, :], comb[:, c * 128:(c + 1) * 128], ident[:])
        cT_sb = ptp.tile([128, 2, 128], BF, tag="ctsb")
        nc.vector.tensor_copy(cT_sb[:], cT_ps[:, 0:2, :])

        fin = ps_tile()[:, 0:D_IN]
        for c in range(2):
            nc.tensor.matmul(fin[:], lhsT=cT_sb[:, c, :], rhs=w_sb["o"][:, c, :],
                             start=(c == 0), stop=(modes[4] != 2 and c == 1))
        if modes[4] == 2:
            nc.tensor.matmul(fin[:], lhsT=ones_row[:, 0:128], rhs=brow_sb[:, 4, :],
                             start=False, stop=True)
        out_sb = sing.tile([128, D_IN], F32)
        nc.scalar.activation(out_sb[:], fin[:], AF.Copy, scale=rowm_sb[:, 0:1])
        nc.sync.dma_start(out=out[:, :], in_=out_sb[0:qpad, :])

    # Steer insert_act_table_loads to the one set covering Square/Ln/Exp/Copy
    orig_tables = bacc.get_activation_tables
    keep = "natural_log_exp_and_others"

    def _patched(arch):
        t = orig_tables(arch)
        return {name: (fs if name == keep else set()) for name, fs in t.items()}

    bacc.get_activation_tables = _patched
    try:
        nc.compile()
    finally:
        bacc.get_activation_tables = orig_tables
    return nc


def _prep_common(inputs):
    ln_in_g = np.asarray(inputs["ln_in_g"], np.float64)
    ln_in_b = np.asarray(inputs["ln_in_b"], np.float64)
    ln_b_g = np.asarray(inputs["ln_b_g"], np.float64)
    Wq = np.asarray(inputs["Wq"], np.float64)
    Wk = np.asarray(inputs["Wk"], np.float64)
    Wv = np.asarray(inputs["Wv"], np.float64)
    Wg = np.asarray(inputs["Wg"], np.float64)
    Wb = np.asarray(inputs["Wb"], np.float64)
    Wo = np.asarray(inputs["Wo"], np.float64)
    bg = np.asarray(inputs["bg"], np.float64)
    bo = np.asarray(inputs["bo"], np.float64)

    def arr_w(w):  # [256, 256] -> [128, 2, 256] din-chunk grouping
        return np.ascontiguousarray(
            w.reshape(2, 128, D_IN).transpose(1, 0, 2)).astype(BF16)

    wall = np.ascontiguousarray(np.stack([
        arr_w(Wq * ln_in_g[:, None]),
        arr_w(Wk * ln_in_g[:, None] * SCALE),
        arr_w(Wv * ln_in_g[:, None]),
        arr_w(Wg * ln_in_g[:, None]),
        arr_w(Wo),
    ], axis=1))

    brow_rows = [
        ln_in_b @ Wq,
        (ln_in_b @ Wk) * SCALE,
        ln_in_b @ Wv,
        ln_in_b @ Wg + bg,
        bo,
    ]
    brows = np.stack(brow_rows).astype(BF16)

    modes = []
    for i, r in enumerate(brow_rows):
        if np.all(r == 0.0):
            modes.append(0)
        elif i == 3 and np.all(r == r[0]):
            modes.append(1)  # uniform gate bias -> activation bias operand
        else:
            modes.append(2)
    gbias_val = -float(brow_rows[3][0]) if modes[3] == 1 else 0.0
    gbias = np.full((128, 1), gbias_val, np.float32)

    c1 = ln_b_g @ Wb                        # [H]
    wext = np.zeros((D_BIAS, 16), np.float64)
    wext[:, 0:H] = Wb * ln_b_g[:, None] - c1[None, :] / D_BIAS
    wext[:, 8] = 1.0 / D_BIAS
    wext = wext.astype(BF16)

    return dict(wall=wall, brows=brows, wext=wext, gbias=gbias), tuple(modes)


def _plan(mask):
    """Compaction plan: per-batch unmasked key indices (uniform kpad) and
    per-core query index groups (uniform qpad)."""
    kidx = [np.where(mask[b] != 0)[0] for b in range(B)]
    kmax = max(len(k) for k in kidx)
    kpad = max(32, -(-kmax // 32) * 32)
    qgroups = []
    for b in range(B):
        qgroups.extend(np.array_split(kidx[b], 4))
    qmax = max(len(q) for q in qgroups)
    qpad = max(32, -(-qmax // 8) * 8)   # DMA slices stay contiguous for any qpad
    return kidx, kpad, qgroups, qpad


def _make_in_maps(inputs):
    x = np.asarray(inputs["x"], np.float32)
    bias = np.asarray(inputs["bias"], np.float32)
    mask = np.asarray(inputs["mask"])
    common, modes = _prep_common(inputs)
    kidx, kpad, qgroups, qpad = _plan(mask)
    nkr = (kpad + 127) // 128
    kfull = nkr * 128

    in_maps = []
    for c in range(8):
        b = c // 4
        ks, qs = kidx[b], qgroups[c]
        K, Q = len(ks), len(qs)
        nat = bias[b][np.ix_(qs, ks)].astype(BF16)       # [Q, K, 128]
        bt = np.zeros((D_BIAS, kpad, qpad), BF16)
        bt[:, :K, :Q] = nat.transpose(2, 1, 0)
        xk = np.zeros((kfull, D_IN), np.float32)
        xk[:K] = x[b, ks]
        xq = np.zeros((qpad, D_IN), np.float32)
        xq[:Q] = x[b, qs]
        mkr = np.zeros((128, kpad), np.float32)
        mkr[:, K:] = NEG
        rowm = np.zeros((128, 1), np.float32)
        rowm[:Q] = 1.0
        in_maps.append(dict(
            bias_tr=np.ascontiguousarray(bt),
            x_k=xk, x_q=xq, mk=mkr, rowm=rowm,
            **common,
        ))
    return in_maps, modes, kpad, qpad


def kernel(**inputs):
    mask = np.asarray(inputs["mask"])
    in_maps, modes, kpad, qpad = _make_in_maps(inputs)
    key = (kpad, qpad, modes)
    if key not in _CACHE:
        _CACHE[key] = _build(kpad, qpad, modes, None)
        _CACHE["last"] = _CACHE[key]
    nc = _CACHE[key]

    res = run_bass_kernel_spmd(nc, in_maps, list(range(8)))
    out = np.zeros((B, L, D_IN), np.float32)
    _, _, qgroups, _ = _plan(mask)
    for c in range(8):
        b = c // 4
        qs = qgroups[c]
        out[b, qs] = res.results[c]["out"][:len(qs)]
    return out
